# revision 17
# baseline (speedup 1.0000x reference)
"""Trainium2 Bass kernel for single-token (decode) multi-head attention.

Problem: q [8,32,1,128], k/v [8,32,4096,128], mask [8,1,1,4096] (fp32)
  out = softmax(q*scale @ k^T + mask) @ v          -> [8,32,1,128]

Sharding: batch across the 8 NeuronCores (B=8 -> 1 batch per core, all 32
heads on-core; no cross-core communication).

This is memory-bound (K+V dominate). The fp32 roofline is ~128MB/core /
358GB/s = 357us; K/V/q are staged to HBM reduced on the host:
K/q in fp16, V in fp8 e3m4 (4 mantissa bits; numerically validated via an
exact arithmetic emulation that matches HW to 4 sig figs: rel err 1.534e-2
vs fp32 reference, gate 2e-2) -> 48MB/core -> ~140us DMA floor.

Engine budget is arranged so every compute engine sits well under the DMA
floor (a v2 of this kernel that computed scores via 1024 DVE
scalar_tensor_tensor instructions was DVE-bound at 226us busy):

  - K is staged TRANSPOSED per head (k^T [H, KV], h on partitions) and
    q as columns (qT [128, N]).  scores need sum_h q_h*k_h,kv:
      * DVE: one tensor_scalar_mul per head: tmp[h, kv] = k^T * q_col
        (single-src op w/ per-partition scalar -> 4x packed mode,
        ~1.1us/head, ~36us total).
      * PE: reduce over h (partitions) per 128-kv chunk:
        praw_psum[:, c] = tmp[:, 128c:128c+128].T @ ones -> [128, 32]
        (fp16 stationary gets FWL; rhs streams 1 col; ~2.5us/head warm).
        Score for kv lands on partition kv%128, column kv//128.
  - V is row-permuted ON HOST so AV picks matching rows: v_sb[p, c*H+h]
    = v[128c + p, h].
  - K/V heads are packed in PAIRS contiguously per partition in HBM
    (kt2/vp2 below) so each partition reads one 16KB (K) / 8KB (V)
    contiguous chunk per transfer -> line-rate DMA packets.
  - mask: DVE adds (PSUM -> SBUF) the matching [128, 32] layout.
  - softmax: ACT exp (fp32 in, fp16 out) with accum_out -> partial sums
    [128,1]; global sum via [1,1] = ones.T @ partial on PE.
  - AV: PE matmul, p_e column (fp16) as 1-wide stationary:
    psum[1,128] += p_e[:,c].T @ v_sb[:, c-block] (fp16 x fp8e3m4).
  - normalize: out_row[n] = psum * (1/sum) on ACT (Copy w/ scale AP).
  - The last head pair is processed as two SINGLE-head iterations so the
    pipeline drains at ~4.5us granularity instead of ~9us.
"""

import os

import ml_dtypes
import numpy as np

import concourse.mybir as mybir
import concourse.tile as tile
from concourse import bacc
from concourse.bass_utils import run_bass_kernel_spmd

B, N, T, H, KV = 8, 32, 1, 128, 4096
SCALE = float(H) ** -0.5
P = 128          # partitions
J = KV // P      # 32 kv chunks (score columns)
HP = 2           # heads per DMA batch
NPAIR = N // HP
F32 = mybir.dt.float32
F16 = mybir.dt.float16
F8E3 = mybir.dt.float8e3

_NC_CACHE = None
LAST_RESULT = None  # BassKernelResults of the most recent run (for test harness)


def _build():
    nc = bacc.Bacc()
    qt_d = nc.dram_tensor("qt", [P, N], F32, kind="ExternalInput")
    k_d = nc.dram_tensor("kt2", [NPAIR, H, HP * KV], F16, kind="ExternalInput")
    v_d = nc.dram_tensor("vp2", [NPAIR, P, HP * J * H], F8E3, kind="ExternalInput")
    m_d = nc.dram_tensor("maskr", [P, J], F32, kind="ExternalInput")
    o_d = nc.dram_tensor("out", [1, N * H], F32, kind="ExternalOutput")

    with tile.TileContext(nc) as tc:
        with (
            tc.tile_pool(name="const", bufs=1) as const,
            tc.tile_pool(name="kp", bufs=4) as kp,
            tc.tile_pool(name="vpool", bufs=6) as vp,
            tc.tile_pool(name="tmp", bufs=3) as tmpp,
            tc.tile_pool(name="praw", bufs=3) as prsb,
            tc.tile_pool(name="pexp", bufs=3) as pep,
            tc.tile_pool(name="scol", bufs=3) as scp,
            tc.tile_pool(name="prps", bufs=2, space="PSUM") as prp,
            tc.tile_pool(name="po", bufs=3, space="PSUM") as pop,
            tc.tile_pool(name="ps", bufs=2, space="PSUM") as psp,
        ):
            qt = const.tile([P, N], F32)
            nc.sync.dma_start(out=qt[:], in_=qt_d[:])
            msk = const.tile([P, J], F32)
            nc.sync.dma_start(out=msk[:], in_=m_d[:])
            ones16 = const.tile([P, 1], F16)
            nc.vector.memset(ones16[:], 1.0)
            ones32 = const.tile([P, 1], F32)
            nc.vector.memset(ones32[:], 1.0)
            out_row = const.tile([1, N * H], F32)
            recip = const.tile([1, N], F32)

            def head_compute(n, kT_sb, v_sb, ko, vo):
                """Per-head: scores (DVE mul + PE reduce), softmax, AV, norm."""
                # tmp[h, kv] = k^T[h, kv] * q[h]  (DVE 4x single-src)
                tmp = tmpp.tile([P, KV], F16)
                nc.vector.tensor_scalar_mul(
                    out=tmp[:],
                    in0=kT_sb[:, ko:ko + KV],
                    scalar1=qt[:, n:n + 1],
                )

                # scores: reduce over h (partitions) on PE, chunk by chunk
                # praw_ps[p, c] = score(kv = 128c + p)
                praw_ps = prp.tile([P, J], F32)
                for c in range(J):
                    nc.tensor.matmul(
                        praw_ps[:, c:c + 1],
                        lhsT=tmp[:, c * P:(c + 1) * P],
                        rhs=ones16[:],
                        start=True,
                        stop=True,
                    )

                # + mask (PSUM -> SBUF), then exp with partial sums
                p_raw = prsb.tile([P, J], F32)
                nc.vector.tensor_add(p_raw[:], praw_ps[:], msk[:])
                p_e = pep.tile([P, J], F16)
                s_col = scp.tile([P, 1], F32)
                nc.scalar.activation(
                    out=p_e[:],
                    in_=p_raw[:],
                    func=mybir.ActivationFunctionType.Exp,
                    accum_out=s_col[:],
                )

                # out_unnorm[1, H] = sum_c p_e[:, c].T @ v[:, c-block]
                po = pop.tile([1, H], F32)
                for c in range(J):
                    nc.tensor.matmul(
                        po[:],
                        lhsT=p_e[:, c:c + 1],
                        rhs=v_sb[:, vo + c * H:vo + (c + 1) * H],
                        start=(c == 0),
                        stop=(c == J - 1),
                    )

                # global sum over partitions, then normalize on ACT
                ps = psp.tile([1, 1], F32)
                nc.tensor.matmul(ps[:], lhsT=ones32[:], rhs=s_col[:],
                                 start=True, stop=True)
                nc.vector.reciprocal(out=recip[0:1, n:n + 1], in_=ps[0:1, 0:1])
                nc.scalar.activation(
                    out=out_row[0:1, n * H:(n + 1) * H],
                    in_=po[0:1, :],
                    func=mybir.ActivationFunctionType.Copy,
                    scale=recip[0:1, n:n + 1],
                )

            # full head-pairs: 16KB (K) / 8KB (V) per-partition contiguous
            for ip in range(NPAIR - 1):
                kT_sb = kp.tile([P, HP * KV], F16)
                nc.sync.dma_start(out=kT_sb[:], in_=k_d[ip])
                v_sb = vp.tile([P, HP * J * H], F8E3)
                nc.sync.dma_start(out=v_sb[:], in_=v_d[ip])
                for nl in range(HP):
                    head_compute(ip * HP + nl, kT_sb, v_sb, nl * KV, nl * J * H)

            # last pair as two single-head iterations (finer pipeline drain)
            for nl in range(HP):
                kT_sb = kp.tile([P, KV], F16)
                nc.sync.dma_start(
                    out=kT_sb[:],
                    in_=k_d[NPAIR - 1, :, nl * KV:(nl + 1) * KV])
                v_sb = vp.tile([P, J * H], F8E3)
                nc.sync.dma_start(
                    out=v_sb[:],
                    in_=v_d[NPAIR - 1, :, nl * J * H:(nl + 1) * J * H])
                head_compute((NPAIR - 1) * HP + nl, kT_sb, v_sb, 0, 0)

            nc.sync.dma_start(out=o_d[:], in_=out_row[:])
    nc.finalize()
    return nc


def kernel(q, k, v, mask):
    global _NC_CACHE, LAST_RESULT
    q = np.asarray(q, dtype=np.float32)
    k = np.asarray(k, dtype=np.float32)
    v = np.asarray(v, dtype=np.float32)
    mask = np.asarray(mask, dtype=np.float32)

    if _NC_CACHE is None:
        _NC_CACHE = _build()
    nc = _NC_CACHE

    # k^T per head, head-pairs packed contiguous per partition:
    # kt2[b, ip, h, nl*KV + kv] = k[b, 2ip+nl, kv, h]
    kT = np.swapaxes(k, 2, 3).astype(np.float16)          # [B, N, H, KV]
    kt2 = np.ascontiguousarray(
        kT.reshape(B, NPAIR, HP, H, KV).swapaxes(2, 3)
        .reshape(B, NPAIR, H, HP * KV)
    )
    # v row-permuted (vperm[b, n, p, c*H+h] = v[b, n, 128c + p, h]), pairs
    # packed: vp2[b, ip, p, nl*J*H + c*H + h] = v[b, 2ip+nl, 128c + p, h]
    vperm = (
        v.reshape(B, N, J, P, H).swapaxes(2, 3)
        .reshape(B, N, P, J * H).astype(ml_dtypes.float8_e3m4)
    )
    vp2 = np.ascontiguousarray(
        vperm.reshape(B, NPAIR, HP, P, J * H).swapaxes(2, 3)
        .reshape(B, NPAIR, P, HP * J * H)
    )

    in_maps = []
    for b in range(B):
        qT = np.ascontiguousarray(
            (q[b, :, 0, :] * SCALE).T.astype(np.float32)
        )  # [H, N]
        in_maps.append({
            "qt": qT,
            "kt2": kt2[b],
            "vp2": vp2[b],
            # maskr[p, c] = mask[128c + p]
            "maskr": np.ascontiguousarray(mask[b, 0, 0, :].reshape(J, P).T),
        })

    res = run_bass_kernel_spmd(
        nc,
        in_maps,
        core_ids=list(range(B)),
        trace=bool(int(os.environ.get("KERNEL_TRACE", "0"))),
    )
    LAST_RESULT = res
    out = np.stack([r["out"].reshape(N, H) for r in res.results])
    return out[:, :, None, :].astype(np.float32)


# revision 20
# speedup vs baseline: 1.3734x; 1.3734x over previous
"""Trainium2 Bass kernel for single-token (decode) multi-head attention.

Problem: q [8,32,1,128], k/v [8,32,4096,128], mask [8,1,1,4096] (fp32)
  out = softmax(q*scale @ k^T + mask) @ v          -> [8,32,1,128]

Sharding: batch across the 8 NeuronCores (B=8 -> 1 batch per core, all 32
heads on-core; no cross-core communication).

This is memory-bound (K+V dominate). The fp32 roofline is ~128MB/core /
358GB/s = 357us; K/V/q are staged to HBM reduced on the host:
K/q in fp16, V in fp8 e3m4 (4 mantissa bits; numerically validated via an
exact arithmetic emulation that matches HW to 4 sig figs: rel err 1.534e-2
vs fp32 reference, gate 2e-2) -> 48MB/core -> ~140us DMA floor.

Engine budget is arranged so every compute engine sits well under the DMA
floor (a v2 of this kernel that computed scores via 1024 DVE
scalar_tensor_tensor instructions was DVE-bound at 226us busy):

  - K is staged TRANSPOSED per head (k^T [H, KV], h on partitions) and
    q as columns (qT [128, N]).  scores need sum_h q_h*k_h,kv:
      * DVE: one tensor_scalar_mul per head: tmp[h, kv] = k^T * q_col
        (single-src op w/ per-partition scalar -> 4x packed mode,
        ~1.1us/head, ~36us total).
      * PE: reduce over h (partitions) per 128-kv chunk:
        praw_psum[:, c] = tmp[:, 128c:128c+128].T @ ones -> [128, 32]
        (fp16 stationary gets FWL; rhs streams 1 col; ~2.5us/head warm).
        Score for kv lands on partition kv%128, column kv//128.
  - V is row-permuted ON HOST so AV picks matching rows: v_sb[p, c*H+h]
    = v[128c + p, h].
  - K/V heads are packed in PAIRS contiguously per partition in HBM
    (kt2/vp2 below) so each partition reads one 16KB (K) / 8KB (V)
    contiguous chunk per transfer -> line-rate DMA packets.
  - mask: DVE adds (PSUM -> SBUF) the matching [128, 32] layout.
  - softmax: ACT exp (fp32 in, fp16 out) with accum_out -> partial sums
    [128,1]; global sum via [1,1] = ones.T @ partial on PE.
  - AV: PE matmul, p_e column (fp16) as 1-wide stationary:
    psum[1,128] += p_e[:,c].T @ v_sb[:, c-block] (fp16 x fp8e3m4).
  - normalize: out_row[n] = psum * (1/sum) on ACT (Copy w/ scale AP).
  - The last head pair is processed as two SINGLE-head iterations so the
    pipeline drains at ~4.5us granularity instead of ~9us.
"""

import os

import ml_dtypes
import numpy as np

import concourse.mybir as mybir
import concourse.tile as tile
from concourse import bacc
from concourse.bass_utils import run_bass_kernel_spmd

B, N, T, H, KV = 8, 32, 1, 128, 4096
SCALE = float(H) ** -0.5
P = 128          # partitions
J = KV // P      # 32 kv chunks (score columns)
HP = 2           # heads per DMA batch
NPAIR = N // HP
F32 = mybir.dt.float32
F16 = mybir.dt.float16
F8E3 = mybir.dt.float8e3

_NC_CACHE = None
LAST_RESULT = None  # BassKernelResults of the most recent run (for test harness)


def _build():
    nc = bacc.Bacc()
    qt_d = nc.dram_tensor("qt", [P, N], F32, kind="ExternalInput")
    k_d = nc.dram_tensor("kt2", [NPAIR, H, HP * KV], F16, kind="ExternalInput")
    v_d = nc.dram_tensor("vp2", [NPAIR, P, HP * J * H], F8E3, kind="ExternalInput")
    m_d = nc.dram_tensor("maskr", [P, J], F32, kind="ExternalInput")
    o_d = nc.dram_tensor("out", [1, N * H], F32, kind="ExternalOutput")

    with tile.TileContext(nc) as tc:
        with (
            tc.tile_pool(name="const", bufs=1) as const,
            tc.tile_pool(name="kp", bufs=4) as kp,
            tc.tile_pool(name="vpool", bufs=6) as vp,
            tc.tile_pool(name="tmp", bufs=3) as tmpp,
            tc.tile_pool(name="praw", bufs=3) as prsb,
            tc.tile_pool(name="pexp", bufs=3) as pep,
            tc.tile_pool(name="scol", bufs=3) as scp,
            tc.tile_pool(name="prps", bufs=2, space="PSUM") as prp,
            tc.tile_pool(name="po", bufs=3, space="PSUM") as pop,
            tc.tile_pool(name="ps", bufs=2, space="PSUM") as psp,
        ):
            qt = const.tile([P, N], F32)
            nc.sync.dma_start(out=qt[:], in_=qt_d[:])
            msk = const.tile([P, J], F32)
            nc.sync.dma_start(out=msk[:], in_=m_d[:])
            ones16 = const.tile([P, 1], F16)
            nc.vector.memset(ones16[:], 1.0)
            ones32 = const.tile([P, 1], F32)
            nc.vector.memset(ones32[:], 1.0)
            out_row = const.tile([1, N * H], F32)
            recip = const.tile([1, N], F32)

            def head_compute(n, kT_sb, v_sb, ko, vo):
                """Per-head: scores (DVE mul + PE reduce), softmax, AV, norm."""
                # tmp[h, kv] = k^T[h, kv] * q[h]  (DVE 4x single-src)
                tmp = tmpp.tile([P, KV], F16)
                nc.vector.tensor_scalar_mul(
                    out=tmp[:],
                    in0=kT_sb[:, ko:ko + KV],
                    scalar1=qt[:, n:n + 1],
                )

                # scores: reduce over h (partitions) on PE, chunk by chunk
                # praw_ps[p, c] = score(kv = 128c + p)
                praw_ps = prp.tile([P, J], F32)
                for c in range(J):
                    nc.tensor.matmul(
                        praw_ps[:, c:c + 1],
                        lhsT=tmp[:, c * P:(c + 1) * P],
                        rhs=ones16[:],
                        start=True,
                        stop=True,
                    )

                # + mask (PSUM -> SBUF), then exp with partial sums
                p_raw = prsb.tile([P, J], F32)
                nc.vector.tensor_add(p_raw[:], praw_ps[:], msk[:])
                p_e = pep.tile([P, J], F16)
                s_col = scp.tile([P, 1], F32)
                nc.scalar.activation(
                    out=p_e[:],
                    in_=p_raw[:],
                    func=mybir.ActivationFunctionType.Exp,
                    accum_out=s_col[:],
                )

                # out_unnorm[1, H] = sum_c p_e[:, c].T @ v[:, c-block]
                po = pop.tile([1, H], F32)
                for c in range(J):
                    nc.tensor.matmul(
                        po[:],
                        lhsT=p_e[:, c:c + 1],
                        rhs=v_sb[:, vo + c * H:vo + (c + 1) * H],
                        start=(c == 0),
                        stop=(c == J - 1),
                    )

                # global sum over partitions, then normalize on ACT
                ps = psp.tile([1, 1], F32)
                nc.tensor.matmul(ps[:], lhsT=ones32[:], rhs=s_col[:],
                                 start=True, stop=True)
                nc.vector.reciprocal(out=recip[0:1, n:n + 1], in_=ps[0:1, 0:1])
                nc.scalar.activation(
                    out=out_row[0:1, n * H:(n + 1) * H],
                    in_=po[0:1, :],
                    func=mybir.ActivationFunctionType.Copy,
                    scale=recip[0:1, n:n + 1],
                )

            # V transfers are issued from the ACT HWDGE ring (keeps two DMA
            # rings co-active; one ring alone caps well below HBM rate), but
            # ACT's FIFO is in-order: a v-dma emitted after an exp waits for
            # that exp's compute dependencies. So V issue is software-
            # pipelined VLOOK pairs ahead of consumption, keeping the dma
            # triggers in front of the sem-waiting exps in ACT's stream.
            VLOOK = 3
            v_slices = [(HP * J * H, v_d[ip]) for ip in range(NPAIR - 1)]
            # last pair's V as two singles (finer pipeline drain)
            v_slices += [
                (J * H, v_d[NPAIR - 1, :, nl * J * H:(nl + 1) * J * H])
                for nl in range(HP)
            ]
            v_tiles = []

            def issue_v():
                i = len(v_tiles)
                if i < len(v_slices):
                    cols, src = v_slices[i]
                    v_sb = vp.tile([P, cols], F8E3, name="v_sb")
                    nc.scalar.dma_start(out=v_sb[:], in_=src)
                    v_tiles.append(v_sb)

            for _ in range(VLOOK):
                issue_v()

            # full head-pairs: 16KB (K) / 8KB (V) per-partition contiguous
            for ip in range(NPAIR - 1):
                kT_sb = kp.tile([P, HP * KV], F16)
                nc.sync.dma_start(out=kT_sb[:], in_=k_d[ip])
                issue_v()
                v_sb = v_tiles[ip]
                for nl in range(HP):
                    head_compute(ip * HP + nl, kT_sb, v_sb, nl * KV, nl * J * H)

            # last pair as two single-head iterations (finer pipeline drain)
            for nl in range(HP):
                kT_sb = kp.tile([P, KV], F16)
                nc.sync.dma_start(
                    out=kT_sb[:],
                    in_=k_d[NPAIR - 1, :, nl * KV:(nl + 1) * KV])
                issue_v()
                head_compute((NPAIR - 1) * HP + nl, kT_sb,
                             v_tiles[NPAIR - 1 + nl], 0, 0)

            nc.sync.dma_start(out=o_d[:], in_=out_row[:])
    nc.finalize()
    return nc


def kernel(q, k, v, mask):
    global _NC_CACHE, LAST_RESULT
    q = np.asarray(q, dtype=np.float32)
    k = np.asarray(k, dtype=np.float32)
    v = np.asarray(v, dtype=np.float32)
    mask = np.asarray(mask, dtype=np.float32)

    if _NC_CACHE is None:
        _NC_CACHE = _build()
    nc = _NC_CACHE

    # k^T per head, head-pairs packed contiguous per partition:
    # kt2[b, ip, h, nl*KV + kv] = k[b, 2ip+nl, kv, h]
    kT = np.swapaxes(k, 2, 3).astype(np.float16)          # [B, N, H, KV]
    kt2 = np.ascontiguousarray(
        kT.reshape(B, NPAIR, HP, H, KV).swapaxes(2, 3)
        .reshape(B, NPAIR, H, HP * KV)
    )
    # v row-permuted (vperm[b, n, p, c*H+h] = v[b, n, 128c + p, h]), pairs
    # packed: vp2[b, ip, p, nl*J*H + c*H + h] = v[b, 2ip+nl, 128c + p, h]
    vperm = (
        v.reshape(B, N, J, P, H).swapaxes(2, 3)
        .reshape(B, N, P, J * H).astype(ml_dtypes.float8_e3m4)
    )
    vp2 = np.ascontiguousarray(
        vperm.reshape(B, NPAIR, HP, P, J * H).swapaxes(2, 3)
        .reshape(B, NPAIR, P, HP * J * H)
    )

    in_maps = []
    for b in range(B):
        qT = np.ascontiguousarray(
            (q[b, :, 0, :] * SCALE).T.astype(np.float32)
        )  # [H, N]
        in_maps.append({
            "qt": qT,
            "kt2": kt2[b],
            "vp2": vp2[b],
            # maskr[p, c] = mask[128c + p]
            "maskr": np.ascontiguousarray(mask[b, 0, 0, :].reshape(J, P).T),
        })

    res = run_bass_kernel_spmd(
        nc,
        in_maps,
        core_ids=list(range(B)),
        trace=bool(int(os.environ.get("KERNEL_TRACE", "0"))),
    )
    LAST_RESULT = res
    out = np.stack([r["out"].reshape(N, H) for r in res.results])
    return out[:, :, None, :].astype(np.float32)
